# revision 1
# baseline (speedup 1.0000x reference)
"""NeRF-NGP MLP kernel for Trainium2 (8 NeuronCores, pure data parallel).

Network (bias-free, fp32 reference):
  sigma net: x[:, :32] -> 64 -> 64 -> (1 sigma + 15 geo)
  color net: concat(x[:, 32:48], geo) -> 64 -> 64 -> 64 -> 3
  out = [color(3), sigma(1)]   shape [N, 4]

Device strategy (per core, N_CORE = 262144 points):
  - Activations in layout [channels(partitions), points(free)].
  - Block-diagonal weights: each matmul's lhsT is [[W,0],[0,W]] over the
    128 partitions, so ONE matmul advances TWO 512-point chunks at once
    (chunk A channels on partitions 0:64, chunk B on 64:128).  A
    pair-group is 1024 points; a unit is 2 pair-groups.
  - The concat is algebraically fused away on the host:
      W3  = s2[:,1:] @ c0[16:,:]   (geo path, 64x64)
      W3v = c0[:16,:] at rows 32:48 (views path)
    so  h3 = relu(W3.T @ h2 + W3v.T @ x)   via PSUM accumulation.
  - Final layer (4 outputs) swaps operand roles: activations become the
    STATIONARY operand (lhsT = h5[64ch, 64pts] quadrant tiles) and the
    tiny weight [64, 4] is the moving one, so each matmul emits only 4
    PSUM columns instead of 512.  Sigma is folded in by accumulating
    h2 @ w6b (w6b = [0,0,0, s2[:,0]]) into the same PSUM group.  The L6
    psum collects 8 units before one batched evacuation + DMA.
  - Matmul operands fp16 (1 cyc/col on the PE vs 4 for fp32); PSUM fp32.
  - PSUM evacuation (relu + fp32->fp16) is the throughput bound; it is
    split into two independent single-engine pipelines: the scalar
    engine owns pair-group 0 of every unit, the vector engine owns
    pair-group 1 (gpsimd has no PSUM port).  Each pipeline has its own
    one-bank psum pool so the only cross-engine coupling is the PE.
    Every 7th unit the scalar engine takes pair-group 1's L5 evac to
    even out the per-op cost difference (533+125 DVE vs 427+185 Act).
  - Input is host-pre-transposed so DMA bursts are 1KB-contiguous per
    partition; output is returned blocked and un-blocked on the host.
"""

import numpy as np

import concourse.bacc as bacc
import concourse.mybir as mybir
import concourse.tile as tile
from concourse.bass_utils import run_bass_kernel_spmd

F32 = mybir.dt.float32
F16 = mybir.dt.float16
RELU = mybir.ActivationFunctionType.Relu

N_PTS = 2097152
N_CORES = 8
N_CORE = N_PTS // N_CORES      # 262144
T = 512                        # points per chunk = one PSUM bank of fp32
PAIR = 2 * T                   # pair-group (1024 pts)
U = N_CORE // (2 * PAIR)       # 128 units (2048 pts each) per core
SBU = 8                        # units per output superblock
ILV = 4                        # units software-pipelined together
L5SWAP = 9                     # every Nth unit Act takes pg1's L5 evac

# weight free-dim offsets inside the [128, 776] weight tile
WCOL = {"W1": 0, "W2": 128, "W3": 256, "W3v": 384, "W4": 512, "W5": 640,
        "W6a": 768, "W6b": 772}
WFREE = 776

_PROG = {}


def _build_program(u_count):
    nc = bacc.Bacc()
    # per unit: rows = (chunk, ch) = 96, cols = (pair-group, pt) = 1024
    xp = nc.dram_tensor("xp", [u_count, 96, PAIR], F16, kind="ExternalInput")
    wt = nc.dram_tensor("wt", [128, WFREE], F16, kind="ExternalInput")
    od = nc.dram_tensor("od", [u_count // SBU, 128, SBU * 64], F16,
                        kind="ExternalOutput")

    with tile.TileContext(nc) as tc:
        with (
            tc.tile_pool(name="wp", bufs=1) as wp,
            tc.tile_pool(name="xtp", bufs=ILV + 2) as xtp,
            tc.tile_pool(name="h1p", bufs=ILV + 1) as h1p,
            tc.tile_pool(name="h2p", bufs=ILV + 2) as h2p,
            tc.tile_pool(name="h3p", bufs=ILV + 1) as h3p,
            tc.tile_pool(name="h4p", bufs=ILV + 1) as h4p,
            tc.tile_pool(name="h5p", bufs=ILV + 1) as h5p,
            tc.tile_pool(name="osp", bufs=2) as osp,
            tc.tile_pool(name="pa", bufs=4, space="PSUM") as pa,
            tc.tile_pool(name="pb", bufs=3, space="PSUM") as pb,
            tc.tile_pool(name="p6p", bufs=1, space="PSUM") as p6p,
        ):
            w = wp.tile([128, WFREE], F16)
            nc.sync.dma_start(out=w, in_=wt[:, :])

            st = {}
            HPOOL = [h1p, h2p, h3p, h4p, h5p]
            LW = {0: ("W1", 96, None), 1: ("W2", 128, None),
                  2: ("W3", 128, ("W3v", 96)), 3: ("W4", 128, None),
                  4: ("W5", 128, None)}

            def act_relu(h, ps, c0, c1):
                nc.scalar.activation(h[:, c0:c1], ps[:, 0:c1 - c0], RELU)

            def dve_relu(h, ps, c0, c1):
                nc.vector.tensor_scalar_max(h[:, c0:c1], ps[:, 0:c1 - c0], 0.0)

            def layer_step(s, L):
                wname, krows, extra = LW[L]
                h = HPOOL[L].tile([128, PAIR], F16)
                prev = s["hs"][L - 1] if L > 0 else None
                for i in (0, 1):
                    pool = pa if i == 0 else pb
                    ps = pool.tile([128, T], F32)
                    if L == 0:
                        rhs = s["xt"][:, i * T: (i + 1) * T]
                    else:
                        rhs = prev[:, i * T: (i + 1) * T]
                    nc.tensor.matmul(
                        out=ps[:, :],
                        lhsT=w[0:krows, WCOL[wname]: WCOL[wname] + 128],
                        rhs=rhs, start=True, stop=extra is None)
                    if extra is not None:
                        wname2, krows2 = extra
                        nc.tensor.matmul(
                            out=ps[:, :],
                            lhsT=w[0:krows2, WCOL[wname2]: WCOL[wname2] + 128],
                            rhs=s["xt"][:, i * T: (i + 1) * T],
                            start=False, stop=True)
                    f = act_relu if i == 0 else dve_relu
                    if (i == 1 and L == 4 and L5SWAP
                            and s["u"] % L5SWAP == L5SWAP - 1):
                        f = act_relu
                    f(h, ps, i * T, (i + 1) * T)
                s["hs"].append(h)

            def emit_l6(s, u):
                # out[pt64, 4] = h5.T @ c3pad + h2.T @ w6b per 64x64 quadrant
                p6 = s["p6"]
                h5, h2 = s["hs"][4], s["hs"][1]
                base = (u % SBU) * 64
                for pg in (0, 1):
                    for half in (0, 1):
                        rg = 64 * half
                        for wdw in range(8):      # 64-pt windows
                            c0 = base + pg * 32 + wdw * 4
                            cw = pg * T + 64 * wdw
                            out_ap = p6[rg: rg + 64, c0: c0 + 4]
                            nc.tensor.matmul(
                                out=out_ap,
                                lhsT=h5[rg: rg + 64, cw: cw + 64],
                                rhs=w[rg: rg + 64,
                                      WCOL["W6a"]: WCOL["W6a"] + 4],
                                start=True, stop=False,
                                tile_position=(rg, rg))
                            nc.tensor.matmul(
                                out=out_ap,
                                lhsT=h2[rg: rg + 64, cw: cw + 64],
                                rhs=w[rg: rg + 64,
                                      WCOL["W6b"]: WCOL["W6b"] + 4],
                                start=False, stop=True,
                                tile_position=(rg, rg))
                if u == u_count - 2:
                    # split the final superblock's flush so the last DMA is
                    # tiny and starts as early as possible (shorter drain)
                    osb = osp.tile([128, SBU * 64], F16)
                    st[u + 1]["osb_last"] = osb
                    nc.scalar.copy(osb[:, 0: (SBU - 1) * 64],
                                   p6[:, 0: (SBU - 1) * 64])
                    nc.sync.dma_start(
                        out=od[u // SBU][:, 0: (SBU - 1) * 64],
                        in_=osb[:, 0: (SBU - 1) * 64])
                elif u == u_count - 1:
                    osb = s["osb_last"]
                    nc.vector.tensor_copy(
                        osb[:, (SBU - 1) * 64: SBU * 64],
                        p6[:, (SBU - 1) * 64: SBU * 64])
                    nc.sync.dma_start(
                        out=od[u // SBU][:, (SBU - 1) * 64: SBU * 64],
                        in_=osb[:, (SBU - 1) * 64: SBU * 64])
                elif u % SBU == SBU - 1:
                    osb = osp.tile([128, SBU * 64], F16)
                    nc.scalar.copy(osb[:, :], p6[:, :])
                    nc.gpsimd.dma_start(out=od[u // SBU], in_=osb[:, :])

            p6 = None
            for ubase in range(0, u_count, ILV):
                block = range(ubase, ubase + ILV)
                for u in block:
                    xt = xtp.tile([96, PAIR], F16)
                    nc.sync.dma_start(out=xt[:, :], in_=xp[u])
                    if u % SBU == 0:
                        p6 = p6p.tile([128, SBU * 64], F32)
                    st[u] = {"xt": xt, "hs": [], "p6": p6, "u": u}
                for L in range(6):
                    for u in block:
                        if L < 5:
                            layer_step(st[u], L)
                        else:
                            emit_l6(st[u], u)
                for u in block:
                    st.pop(u - ILV, None)

    nc.finalize()
    return nc


def _get_program():
    if "nc" not in _PROG:
        _PROG["nc"] = _build_program(U)
    return _PROG["nc"]


def _block_diag(m):
    out = np.zeros((2 * m.shape[0], 2 * m.shape[1]), np.float32)
    out[: m.shape[0], : m.shape[1]] = m
    out[m.shape[0]:, m.shape[1]:] = m
    return out


def _build_weights(s0, s1, s2, c0, c1, c2, c3):
    w = np.zeros((128, WFREE), np.float32)
    w1 = np.zeros((48, 64), np.float32)
    w1[0:32] = s0
    w[0:96, WCOL["W1"]: WCOL["W1"] + 128] = _block_diag(w1)
    w[0:128, WCOL["W2"]: WCOL["W2"] + 128] = _block_diag(s1)
    w3 = (s2[:, 1:].astype(np.float64) @ c0[16:].astype(np.float64)).astype(
        np.float32)
    w[0:128, WCOL["W3"]: WCOL["W3"] + 128] = _block_diag(w3)
    w3v = np.zeros((48, 64), np.float32)
    w3v[32:48] = c0[:16]
    w[0:96, WCOL["W3v"]: WCOL["W3v"] + 128] = _block_diag(w3v)
    w[0:128, WCOL["W4"]: WCOL["W4"] + 128] = _block_diag(c1)
    w[0:128, WCOL["W5"]: WCOL["W5"] + 128] = _block_diag(c2)
    for rg in (0, 64):
        w[rg: rg + 64, WCOL["W6a"]: WCOL["W6a"] + 3] = c3
        w[rg: rg + 64, WCOL["W6b"] + 3] = s2[:, 0]
    return w


def kernel(x, s0, s1, s2, c0, c1, c2, c3):
    x = np.asarray(x, dtype=np.float32)
    assert x.shape == (N_PTS, 48), x.shape
    args = [np.asarray(a, dtype=np.float32) for a in (s0, s1, s2, c0, c1, c2, c3)]
    w_host = _build_weights(*args).astype(np.float16)

    in_maps = []
    for i in range(N_CORES):
        xc = x[i * N_CORE: (i + 1) * N_CORE]
        # [U units, 2 pair-groups, 2 chunks, T pts, 48 ch]
        #   -> rows (chunk, ch) = 96, cols (pair-group, pt) = 1024
        xprep = np.ascontiguousarray(
            xc.reshape(U, 2, 2, T, 48).transpose(0, 2, 4, 1, 3)
        ).astype(np.float16).reshape(U, 96, PAIR)
        in_maps.append({"xp": xprep, "wt": w_host})

    nc = _get_program()
    res = run_bass_kernel_spmd(nc, in_maps, core_ids=list(range(N_CORES)))

    outs = []
    for i in range(N_CORES):
        od = res.results[i]["od"]          # [U//SBU, 128, SBU*64] f16
        # partition = (half, pt-in-64-window); cols = (unit, pg, wdw8, ch)
        o = od.reshape(U // SBU, 2, 64, SBU, 2, 8, 4)
        o = o.transpose(0, 3, 4, 1, 5, 2, 6)  # [sb, s, pg, half, wdw, p, ch]
        outs.append(o.reshape(N_CORE, 4).astype(np.float32))
    return np.concatenate(outs, axis=0)

